# revision 42
# baseline (speedup 1.0000x reference)
"""Causal self-attention Trainium2 kernel (B=2, T=2048, D=1024, H=16).

Tensor-parallel over heads: each of 8 cores owns 2 heads. Host pre-tiles
x into block-major [block, p, dt*512] (8KB DMA lines); per-core weight
slices; partial outputs summed on host. ~197us vs 213us baseline.

Design (trace-driven):
  - all PE operand streams bf16 (f32r moving operands stream ~370ns per
    N=512 matmul vs 216ns bf16; PSUM accumulation stays f32).
  - S matmul pair row-tiled (h0 rows 0-63 / h1 64-127) -> concurrent on
    the PE (80/80 pairs at ~3ns start offset in the trace).
  - V transposed on the PE into a bf16 view of an f32 PSUM tile; ones
    columns ride the O matmul (M=65) so softmax sums are free.
  - attention starts right after the first 512-token QKV block; all
    remaining QKV / out-projection work is a filler-chunk queue popped
    at q-block boundaries (3) and every other k-tile (every k-tile in
    b1), with xt DMA prefetched one block ahead on the sync queue.
  - normalize: one ACT copy grabs both heads' sums rows from PSUM, DVE
    reciprocal, one gpsimd partition_broadcast, two DVE multiplies that
    replace the PSUM->SBUF casts.
  - last q-block (b1 jq3) ships unnormalized per-head projections
    (row-packed matmul pairs) + sums; host applies 1/sums (linear,
    commutes with the projection) - removes the final normalize chain.
  - outputs are block-tiled [block, p, ot, 512] (2KB DMA lines), host
    reassembles; proj casts split DVE/ACT to pipeline the drain.
"""

import numpy as np

B, T, D = 2, 2048, 1024
H, HD = 16, 64
NCORES = 8
BT = B * T                 # 4096
NQB = T // 512             # q blocks per batch = 4
NDT = D // 128             # contraction d-tiles = 8

_CACHE = {}


def _build():
    import concourse.bass as bass
    import concourse.mybir as mybir
    import concourse.tile as tile
    from concourse import bacc

    f32 = mybir.dt.float32
    bf16 = mybir.dt.bfloat16
    u32 = mybir.dt.uint32
    Exp = mybir.ActivationFunctionType.Exp
    is_ge = mybir.AluOpType.is_ge

    nc = bacc.Bacc(trn_type="TRN2")

    # xTt: block-tiled activations [block, p, dt*512] -> 8KB DMA lines
    xTt = nc.dram_tensor("xTt", [B * NQB, 128, NDT * 512], bf16,
                         kind="ExternalInput")
    wqk = nc.dram_tensor("wqk", [128, NDT, 4 * HD], bf16, kind="ExternalInput")
    wv = nc.dram_tensor("wv", [128, NDT, 2 * HD], bf16, kind="ExternalInput")
    wo = nc.dram_tensor("wo", [2 * HD, D], bf16, kind="ExternalInput")
    id128 = nc.dram_tensor("id128", [128, 128], bf16, kind="ExternalInput")
    # yTt: block-tiled output partials [block, p, ot, 512] -> 2KB lines
    yTt = nc.dram_tensor("yTt", [B * NQB, 128, 8, 512], bf16,
                         kind="ExternalOutput")
    y2 = nc.dram_tensor("y2", [128, 8, 512], bf16, kind="ExternalOutput")
    sums2 = nc.dram_tensor("sums2", [2, 512], f32, kind="ExternalOutput")

    with tile.TileContext(nc) as tc:
        with tc.tile_pool(name="const", bufs=1) as const, \
             tc.tile_pool(name="xt", bufs=2) as xt_pool, \
             tc.tile_pool(name="big", bufs=2) as big, \
             tc.tile_pool(name="sm", bufs=4) as sm, \
             tc.tile_pool(name="ps", bufs=1, space="PSUM") as ps:

            # --- persistent constants. wqk d-slices go on the scalar queue
            # while the first token block's xt d-slices go on sync, so the
            # first QKV matmul group can start ~1.5us in and stream along
            # with the DMAs. wv/wo/wo2 arrive in parallel on gpsimd.
            # warm tile memset on the (otherwise idle) DVE so the warm-up
            # matmuls never wait behind slow DMA-issue instructions
            warm = const.tile([128, 512], bf16)
            nc.vector.memset(warm.bitcast(u32), 0)
            wqk_sb = const.tile([128, NDT, 4 * HD], bf16)
            nc.scalar.dma_start(out=wqk_sb, in_=wqk.ap())
            xt0 = xt_pool.tile([128, NDT, 512], bf16, tag="xt", bufs=3, name="xt")
            nc.sync.dma_start(out=xt0,
                              in_=xTt.ap()[0].rearrange("p (dt c) -> p dt c", dt=NDT))
            wv_sb = const.tile([128, NDT, 2 * HD], bf16)
            nc.gpsimd.dma_start(out=wv_sb, in_=wv.ap())
            wo_sb = const.tile([128, D], bf16)
            nc.gpsimd.dma_start(out=wo_sb, in_=wo.ap())
            id_sb = const.tile([128, 128], bf16)
            nc.gpsimd.dma_start(out=id_sb, in_=id128.ap())

            # --- warm-up: PE activity with no DMA deps flips the HAM clock
            # gate to 2.4GHz before real work lands; dummy exp preloads the
            # ACT table so the 1.3us table load isn't paid mid-attention
            wacc = ps.tile([128, 512], f32, tag="mmA", bufs=2, name="wacc")
            for i in range(14):
                nc.tensor.matmul(wacc, warm[:, 0:128], warm,
                                 start=True, stop=True, skip_group_check=True)
            wexp = const.tile([1, 2], f32)
            nc.scalar.activation(wexp, warm[0:1, 0:2], Exp, scale=1.0)

            state = {}
            DRAIN = {"on": False}

            xts = {}
            pf_planned = {(0, 0)}

            def qkv_chunks(b, jts, first=False, pf_next=None):
                """QKV projection for batch b, token blocks jts, as fillers.

                xt DMAs are issued by separate prefetch chunks one block
                ahead, so popped matmul chunks never stall on HBM.
                """
                t0 = b * T
                if first:
                    class _C:
                        tensor_copy = staticmethod(nc.scalar.copy)
                    ceng = _C
                else:
                    ceng = nc.vector
                if b not in state:
                    Q2 = big.tile([128, T], bf16, tag="Q2", bufs=2, name="Q2")
                    K2 = big.tile([128, T], bf16, tag="K2", bufs=2, name="K2")
                    v_sb = big.tile([128, 16, 256], bf16, tag="v", bufs=2, name="v_sb")
                    state[b] = {"Q2": Q2, "K2": K2, "v": v_sb}
                st = state[b]
                Q2, K2, v_sb = st["Q2"], st["K2"], st["v"]

                def c_pf(b, jt):
                    xt = xt_pool.tile([128, NDT, 512], bf16, tag="xt",
                                      bufs=3, name="xt")
                    xts[(b, jt)] = xt
                    nc.sync.dma_start(
                        out=xt, in_=xTt.ap()[b * NQB + jt]
                        .rearrange("p (dt c) -> p dt c", dt=NDT))

                chunks = []
                if jts[0] == 0:
                    def c_ones():
                        # only the ones columns (64 and 128+64 of each slot)
                        ones_view = v_sb.rearrange(
                            "p t (g c) -> p t g c", g=2)[:, :, :, 64:65]
                        nc.vector.memset(ones_view, 1.0)
                    chunks.append(c_ones)
                for i, jt in enumerate(jts):
                    if (b, jt) not in pf_planned:
                        pf_planned.add((b, jt))
                        chunks.append(lambda b=b, jt=jt: c_pf(b, jt))
                    def c_load(jt=jt):
                        xt = xts[(b, jt)]
                        st["xt"] = xt
                        acc = ps.tile([128, 512], f32, tag="mmA", bufs=2, name="acc")
                        for d in range(NDT):
                            nc.tensor.matmul(acc, wqk_sb[:, d, 0:128], xt[:, d, :],
                                             start=(d == 0), stop=(d == NDT - 1))
                        ceng.tensor_copy(Q2[:, jt * 512:(jt + 1) * 512], acc)
                    def c_k(jt=jt):
                        xt = st["xt"]
                        acc = ps.tile([128, 512], f32, tag="mmA", bufs=2, name="acc")
                        for d in range(NDT):
                            nc.tensor.matmul(acc, wqk_sb[:, d, 128:256], xt[:, d, :],
                                             start=(d == 0), stop=(d == NDT - 1))
                        ceng.tensor_copy(K2[:, jt * 512:(jt + 1) * 512], acc)
                    def c_v(jt=jt):
                        xt = st["xt"]
                        acc = ps.tile([128, 512], f32, tag="mmA", bufs=2, name="acc")
                        for d in range(NDT):
                            nc.tensor.matmul(acc, wv_sb[:, d, :], xt[:, d, :],
                                             start=(d == 0), stop=(d == NDT - 1))
                        vt = sm.tile([128, 512], bf16, tag="vT", bufs=2, name="vT")
                        ceng.tensor_copy(vt, acc)
                        st["vT"] = vt
                    def c_tr(jt=jt):
                        # V transpose on the PE (bf16, ~230ns each); output
                        # goes into a bf16 view of an f32 PSUM tile, then one
                        # strided cast drops all 4 k-tiles into v_sb slots
                        vt = st["vT"]
                        trp = ps.tile([128, 512], f32, tag="mmA", bufs=2,
                                      name="trp").bitcast(bf16)
                        for c in range(4):
                            nc.tensor.transpose(trp[:, c * 128:(c + 1) * 128],
                                                vt[:, c * 128:(c + 1) * 128],
                                                id_sb)
                        dst = v_sb[:, jt * 4:(jt + 1) * 4, :] \
                            .rearrange("p t (s c) -> p t s c", s=2)[:, :, :, 0:64]
                        src = trp[:, 0:512].rearrange("p (t s c) -> p t s c",
                                                      t=4, s=2)
                        ceng.tensor_copy(dst, src)
                    chunks += [c_load, c_k, c_v, c_tr]
                    # prefetch the following block right after this one's Q
                    nxt = None
                    if i + 1 < len(jts):
                        nxt = (b, jts[i + 1])
                    elif pf_next is not None:
                        nxt = pf_next
                    if nxt is not None and nxt not in pf_planned:
                        pf_planned.add(nxt)
                        chunks.insert(len(chunks) - 3,
                                      lambda bb=nxt[0], jj=nxt[1]: c_pf(bb, jj))
                return chunks

            def proj_chunks(b, jq):
                """Out-projection of q-block jq (after its normalize)."""
                st = state[b]
                qs = slice(jq * 512, (jq + 1) * 512)
                yT_dst = yTt.ap()[b * NQB + jq]
                chunks = []
                for og in range(4):
                    def c_proj(og=og, st=st):
                        O_stack = st["Ost"]
                        if og == 0:
                            st["ysb"] = sm.tile([128, 8, 512], bf16, tag="ysb",
                                                bufs=2, name="ysb")
                        ysb = st["ysb"]
                        if DRAIN["on"]:
                            ypt = ps.tile([128, 1024], f32, tag="S", bufs=2,
                                          name="ypt")
                        for i, ot in enumerate((2 * og, 2 * og + 1)):
                            if DRAIN["on"]:
                                yp = ypt[:, 512 * i:512 * (i + 1)]
                            else:
                                yp = ps.tile([128, 512], f32, tag="mmA",
                                             bufs=2, name="yp")
                            nc.tensor.matmul(yp, wo_sb[:, ot * 128:(ot + 1) * 128],
                                             O_stack[:, qs], start=True, stop=True,
                                             skip_group_check=True)
                            if ot % 2:
                                nc.scalar.copy(ysb[:, ot, :], yp)
                            else:
                                nc.vector.tensor_copy(ysb[:, ot, :], yp)
                        eng = nc.gpsimd if og % 2 else nc.sync
                        eng.dma_start(out=yT_dst[:, 2 * og:2 * og + 2, :],
                                      in_=ysb[:, 2 * og:2 * og + 2, :])
                    chunks.append(c_proj)
                return chunks

            def tail_proj_chunks(b, jq):
                """Final q-block: project UNNORMALIZED per-head O (row-packed
                pair) and ship partials + sums; the host applies 1/sums.
                Removes the final normalize chain from the critical path."""
                st = state[b]
                qs = slice(jq * 512, (jq + 1) * 512)
                yT_dst = yTt.ap()[b * NQB + jq]
                y2_dst = y2.ap()
                chunks = []
                for og in range(4):
                    def c_proj(og=og, st=st):
                        O_stack = st["Ost"]
                        if og == 0:
                            st["ysb"] = sm.tile([128, 8, 512], bf16, tag="ysb",
                                                bufs=2, name="ysb")
                            st["ysb2"] = sm.tile([128, 8, 512], bf16, tag="ysb2",
                                                 bufs=2, name="ysb2")
                        ysb, ysb2 = st["ysb"], st["ysb2"]
                        # attention is over: the S banks are free, so the
                        # row-packed pairs spread over S + O/mmA banks and
                        # the 16 matmuls never wait on a cast
                        ypt = ps.tile([128, 1024], f32, tag="S", bufs=2,
                                      name="ypt")
                        if og % 2:
                            yp2t = ps.tile([128, 1024], f32, tag="O", bufs=1,
                                           name="yp2t")
                        else:
                            yp2t = None
                        for i, ot in enumerate((2 * og, 2 * og + 1)):
                            osl = slice(ot * 128, (ot + 1) * 128)
                            yp = ypt[:, 512 * i:512 * (i + 1)]
                            nc.tensor.matmul(yp, wo_sb[0:64, osl],
                                             O_stack[0:64, qs], start=True,
                                             stop=True, skip_group_check=True)
                            if yp2t is not None:
                                yp2 = yp2t[:, 512 * i:512 * (i + 1)]
                            else:
                                yp2 = ps.tile([128, 512], f32, tag="mmA",
                                              bufs=2, name="yp2")
                            nc.tensor.matmul(yp2, wo_sb[64:128, osl],
                                             O_stack[64:128, qs], start=True,
                                             stop=True, skip_group_check=True)
                            nc.vector.tensor_copy(ysb[:, ot, :], yp)
                            nc.scalar.copy(ysb2[:, ot, :], yp2)
                        eng = nc.gpsimd if og % 2 else nc.sync
                        eng.dma_start(out=yT_dst[:, 2 * og:2 * og + 2, :],
                                      in_=ysb[:, 2 * og:2 * og + 2, :])
                        eng2 = nc.sync if og % 2 else nc.gpsimd
                        eng2.dma_start(out=y2_dst[:, 2 * og:2 * og + 2, :],
                                       in_=ysb2[:, 2 * og:2 * og + 2, :])
                    chunks.append(c_proj)
                return chunks

            def attn_emit(b, fillers, jqs, tail_jq=None):
                st = state[b]
                Q2, K2, v_sb = st["Q2"], st["K2"], st["v"]
                O_stack = big.tile([128, T], bf16, tag="Ost", bufs=2, name="Ost")
                tmp1 = big.tile([64, T], bf16, tag="tmp1", bufs=2, name="tmp1")
                st["Ost"] = O_stack
                st["tmp1"] = tmp1
                pending = list(fillers)
                for jq in jqs:
                    nk = 4 * (jq + 1)
                    tail = (jq == tail_jq)
                    Opair = ps.tile([128, 1024], f32, tag="O", bufs=1,
                                    name="O")[0:65, :]
                    qs = slice(jq * 512, (jq + 1) * 512)

                    def emit_S(ik):
                        """Row-tiled S matmul pair + exp+mask -> pt."""
                        ks = slice(ik * 128, (ik + 1) * 128)
                        S = ps.tile([128, 1024], f32, tag="S", bufs=2, name="S")
                        for h in range(2):
                            pr = slice(64 * h, 64 * (h + 1))
                            nc.tensor.matmul(S[:, 512 * h:512 * (h + 1)],
                                             K2[pr, ks], Q2[pr, qs],
                                             start=True, stop=True,
                                             skip_group_check=True)
                        pt = sm.tile([128, 1024], bf16, tag="pT", bufs=3, name="pT")
                        d = (ik - 4 * jq) * 128
                        if d < 0:
                            nc.scalar.activation(pt, S, Exp, scale=0.125)
                        else:
                            pt_v = pt.rearrange("p (h c) -> p h c", h=2)
                            S_v = S.rearrange("p (h c) -> p h c", h=2)
                            if d > 0:
                                nc.gpsimd.memset(
                                    pt_v[:, :, 0:d].bitcast(u32), 0)
                            nc.scalar.activation(pt_v[:, :, d:512], S_v[:, :, d:512],
                                                 Exp, scale=0.125)
                            tri = pt_v[:, :, d:d + 128]
                            nc.gpsimd.affine_select(
                                tri, tri, pattern=[[0, 2], [1, 128]],
                                compare_op=is_ge, fill=0.0,
                                channel_multiplier=-1, base=0)
                        return pt

                    def emit_O(ik, pt):
                        for h in range(2):
                            nc.tensor.matmul(Opair[:, 512 * h:512 * (h + 1)],
                                             v_sb[:, ik, 128 * h:128 * h + 65],
                                             pt[:, 512 * h:512 * (h + 1)],
                                             start=(ik == 0), stop=(ik == nk - 1),
                                             skip_group_check=True)

                    # software-pipelined: S(ik+1) issues before O(ik) so the
                    # in-order PE queue never parks on exp(ik); at the block
                    # start two fillers run before O(0) so the previous
                    # block's normalize chain never stalls the PE
                    prev = None
                    for ik in range(nk):
                        pt = emit_S(ik)
                        if ik == 1:
                            for _ in range(3):
                                if pending:
                                    pending.pop(0)()
                        if prev is not None:
                            emit_O(*prev)
                            if (b == 1 or ik % 2) and ik > 1 and pending:
                                pending.pop(0)()
                        prev = (ik, pt)
                    emit_O(*prev)
                    if tail:
                        # ship sums + unnormalized per-head O; host fixes up
                        for h in range(2):
                            Oh = Opair[:, 512 * h:512 * (h + 1)]
                            srow = sm.tile([1, 512], f32, tag="srow", bufs=4,
                                           name="srow")
                            nc.scalar.copy(srow, Oh[64:65, :])
                            nc.gpsimd.dma_start(out=sums2.ap()[h:h + 1, :],
                                                in_=srow)
                            dst = O_stack[0:64, qs] if h == 0 else tmp1[:, qs]
                            nc.vector.tensor_copy(dst, Oh[0:64, :])
                        nc.gpsimd.dma_start(out=O_stack[64:128, qs],
                                            in_=tmp1[:, qs])
                        pending = tail_proj_chunks(b, jq) + pending
                    else:
                        # normalize q-block jq: O / sums. Per-head chains
                        # with the sums-row copies split across ACT and DVE
                        # so the h0 chain starts immediately and Opair frees
                        # ~1.5us earlier at every exposed boundary.
                        for h in range(2):
                            Oh = Opair[:, 512 * h:512 * (h + 1)]
                            srow = sm.tile([1, 512], f32, tag="srow2", bufs=4,
                                           name="srow2")
                            if h == 0:
                                nc.scalar.copy(srow, Oh[64:65, :])
                            else:
                                nc.vector.tensor_copy(srow, Oh[64:65, :])
                            rrow = sm.tile([1, 512], f32, tag="rrow", bufs=4,
                                           name="rrow")
                            nc.vector.reciprocal_approx_fast(rrow, srow)
                            Bt = sm.tile([64, 512], f32, tag="B", bufs=4,
                                         name="B")
                            nc.gpsimd.partition_broadcast(Bt, rrow)
                            dst = O_stack[0:64, qs] if h == 0 else tmp1[:, qs]
                            nc.vector.tensor_tensor(dst, Oh[0:64, :], Bt,
                                                    mybir.AluOpType.mult)
                        nc.gpsimd.dma_start(out=O_stack[64:128, qs],
                                            in_=tmp1[:, qs])
                        pending.extend(proj_chunks(b, jq))
                return pending

            xts[(0, 0)] = xt0
            # prologue: first 512-token block of b0 inline + prefetch of
            # block 1, so the first popped filler never stalls on HBM;
            # attention starts immediately after
            for c in qkv_chunks(0, [0], first=True, pf_next=(0, 1)):
                c()
            fillers = qkv_chunks(0, [1, 2, 3], pf_next=(1, 0)) \
                + qkv_chunks(1, [0, 1, 2, 3])
            pending = attn_emit(0, fillers, [0, 1, 2, 3])
            # b1 ascending; the last (biggest) block drains leftover filler
            # chunks during its 16 k-tiles and ships unnormalized partials
            pending = attn_emit(1, pending, [0, 1, 2, 3], tail_jq=3)
            DRAIN["on"] = True
            while pending:
                pending.pop(0)()

    nc.compile()
    return nc


def _prep_inputs(x, W_qkv, W_out):
    """Host-side shard prep. Returns per-core input maps."""
    x = np.ascontiguousarray(x, dtype=np.float32)
    W_qkv = np.ascontiguousarray(W_qkv, dtype=np.float32)
    W_out = np.ascontiguousarray(W_out, dtype=np.float32)

    from ml_dtypes import bfloat16
    # block-tiled xT: [block, p, dt*512]; row (bj, p) holds
    # x[bj*512:(bj+1)*512, dt*128+p] for dt = 0..7 -> 8KB DMA lines
    xf = x.reshape(BT, D)
    xTt = np.ascontiguousarray(
        xf.reshape(B * NQB, 512, NDT, 128).transpose(0, 3, 2, 1)
        .reshape(B * NQB, 128, NDT * 512)).astype(bfloat16)
    Wq = W_qkv[0 * D:1 * D]
    Wk = W_qkv[1 * D:2 * D]
    Wv = W_qkv[2 * D:3 * D]

    in_maps = []
    for c in range(NCORES):
        h0, h1 = 2 * c, 2 * c + 1
        wqk = np.concatenate([
            Wq[h0 * HD:(h0 + 1) * HD].T, Wq[h1 * HD:(h1 + 1) * HD].T,
            Wk[h0 * HD:(h0 + 1) * HD].T, Wk[h1 * HD:(h1 + 1) * HD].T], axis=1)
        wv = np.concatenate([
            Wv[h0 * HD:(h0 + 1) * HD].T, Wv[h1 * HD:(h1 + 1) * HD].T], axis=1)
        wo = np.concatenate([
            W_out[:, h0 * HD:(h0 + 1) * HD].T, W_out[:, h1 * HD:(h1 + 1) * HD].T], axis=0)
        # weights pre-tiled per d-slice: [p, dt, cols], contiguous lines
        wqk_t = np.ascontiguousarray(
            wqk.reshape(NDT, 128, 4 * HD).transpose(1, 0, 2)).astype(bfloat16)
        wv_t = np.ascontiguousarray(
            wv.reshape(NDT, 128, 2 * HD).transpose(1, 0, 2)).astype(bfloat16)
        in_maps.append({
            "xTt": xTt,
            "wqk": wqk_t,
            "wv": wv_t,
            "wo": np.ascontiguousarray(wo).astype(bfloat16),
            "id128": np.eye(128, dtype=np.float32).astype(bfloat16),
        })
    return in_maps


def kernel(x, W_qkv, W_out):
    from concourse.bass_utils import run_bass_kernel_spmd

    if "nc" not in _CACHE:
        _CACHE["nc"] = _build()
    nc = _CACHE["nc"]

    in_maps = _prep_inputs(np.asarray(x), np.asarray(W_qkv), np.asarray(W_out))
    res = run_bass_kernel_spmd(nc, in_maps, core_ids=list(range(NCORES)))
    _CACHE["last_results"] = res

    yT = np.zeros((D, BT), dtype=np.float32)
    for r in res.results:
        # yTt [block, p, ot, 512] -> [D, BT]
        yt = r["yTt"].astype(np.float32)          # [8, 128, 8, 512]
        ytf = yt.transpose(2, 1, 0, 3).reshape(D, BT)
        yT += ytf
        # tail block (b1 jq0 = block 4): unnormalized h0 in yTt, h1 in y2
        r0 = 1.0 / r["sums2"][0]
        r1 = 1.0 / r["sums2"][1]
        t4 = ytf[:, 4 * 512:5 * 512]
        y2f = r["y2"].astype(np.float32).transpose(1, 0, 2).reshape(D, 512)
        yT[:, 4 * 512:5 * 512] += t4 * (r0 - 1.0) + y2f * r1
    return np.ascontiguousarray(yT.T).reshape(B, T, D)
